# revision 1
# baseline (speedup 1.0000x reference)
"""Trainium2 Bass kernel for quantized (AdaPT int8-systolic) 3x3 Conv2d.

Reference computation (see problem):
  amax_x = max(|x|) (global), amax_w = max(|w|)
  qx = clip(round(x * 127/amax_x)), qw likewise  (integer-valued)
  out = conv2d(qx, qw, pad=1) / ((127/amax_x)*(127/amax_w)) + bias

Sharding: batch N=32 -> 4 images per core across 8 cores (data parallel),
weight/bias replicated, amax_x via AllReduce-max collective.

Per-core layout: partition dim = (image, channel) = 4*32 = 128.
Quantized input kept resident in SBUF as a zero-padded [128, 226*226+2]
bf16 image so every conv tap is a flat-offset read (padding rows/cols
absorb all edge effects, including row-wrap in flattened coordinates).
Conv = 9 accumulating matmuls per (image, 2-row tile): stationary
[32ci, 64co] per tap, moving = flat 452-px slice of the padded image.
Two images of a pair run on disjoint (row, col) PE sub-arrays
(tile_position) and accumulate into one [128, 452] psum bank.
Integer values <= 127 are exact in bf16 and the fp32 psum accumulation
(<2^24) is exact, so the conv matches the int32 reference bit-for-bit.
"""

import os
import sys
import numpy as np
from contextlib import ExitStack

sys.path.insert(0, "/opt/trn_rl_repo")

MAGIC = 12582912.0  # 1.5 * 2^23: adding then subtracting forces RNE-to-int


def build(nimg=4, H=224, W=224, n_cores=8):
    import concourse.bass as bass
    import concourse.mybir as mybir
    import concourse.tile as tile
    from concourse import bacc

    f32 = mybir.dt.float32
    bf16 = mybir.dt.bfloat16
    CI, CO = 32, 64
    HP, WP = H + 2, W + 2
    assert nimg == 4 and H % 2 == 0

    nc = bacc.Bacc()
    x_ext = nc.declare_dram_parameter("x", [nimg, CI, H, W], f32, isOutput=False)
    qw_ext = nc.declare_dram_parameter("qw_stat", [128, 9 * CO], bf16,
                                       isOutput=False)
    sw_ext = nc.declare_dram_parameter("swv", [128, 1], f32, isOutput=False)
    b_ext = nc.declare_dram_parameter("bias", [CO], f32, isOutput=False)
    out_ext = nc.declare_dram_parameter("out", [nimg, CO, H, W], f32, isOutput=True)

    cc_in = nc.dram_tensor("cc_in", [1, 1], f32)
    cc_out = nc.dram_tensor("cc_out", [1, 1], f32)

    AT = mybir.AluOpType
    AF = mybir.ActivationFunctionType

    with ExitStack() as ctx:
        tc = ctx.enter_context(tile.TileContext(nc))

        consts = ctx.enter_context(tc.tile_pool(name="consts", bufs=1))
        chunks = ctx.enter_context(tc.tile_pool(name="chunks", bufs=8))
        tmps = ctx.enter_context(tc.tile_pool(name="tmps", bufs=3))
        xqp = ctx.enter_context(tc.tile_pool(name="xqp", bufs=1))
        statp = ctx.enter_context(tc.tile_pool(name="statp", bufs=1))
        psump = ctx.enter_context(tc.tile_pool(name="psum", bufs=6, space="PSUM"))
        psump1 = ctx.enter_context(tc.tile_pool(name="psum1", bufs=1, space="PSUM"))
        outsp = ctx.enter_context(tc.tile_pool(name="outs", bufs=3))

        # Warm up the collectives firmware with a dummy all-reduce so the
        # real amax all-reduce later isn't hit by one-time startup cost.
        if n_cores > 1:
            warm = consts.tile([1, 1], f32)
            nc.vector.memset(warm[:], 0.0)
            nc.sync.dma_start(cc_in[:, :], warm[:])
            nc.gpsimd.collective_compute(
                "AllReduce", AT.max,
                replica_groups=[list(range(n_cores))],
                ins=[cc_in[:, :].opt()],
                outs=[cc_out[:, :].opt()])

        # ---------------- Phase A: amax of x (streamed) and weight ----------
        xflat = x_ext[:, :, :, :].rearrange("n c h w -> (n c) (h w)")  # [128, H*W]
        RA = 8 if H % 8 == 0 else 2  # rows per amax chunk
        n_amax_chunks = H // RA
        ce = RA * W
        partials = consts.tile([128, n_amax_chunks], f32)
        # PE warm-keeper: sparse dummy matmuls through phase A so the HAM
        # clock gate stays at 8/8 when the real conv matmuls begin. Each is
        # gated on its chunk's DMA so they spread through the phase.
        warm_ps = psump1.tile([128, 512], f32, tag="warm")
        ones_row = consts.tile([1, 128], f32)
        nc.vector.memset(ones_row[:], 1.0)

        last_xt = None
        for k in range(n_amax_chunks):
            xt = chunks.tile([128, ce], f32, tag="chunk")
            nc.gpsimd.dma_start(xt[:], xflat[:, k * ce:(k + 1) * ce])
            nc.vector.tensor_reduce(
                partials[:, k:k + 1], xt[:], axis=mybir.AxisListType.X,
                op=AT.max, apply_absolute_value=True)
            nc.tensor.matmul(warm_ps[:, 0:8], ones_row[:, :], xt[0:1, 0:8],
                             start=True, stop=True)
            last_xt = xt
        # emit the first (pool-depth) phase-B x re-read loads NOW, before the
        # collective: the collective blocks the GpSimd queue, and these loads
        # have no dependency on it — they fill chunk slots while it runs.
        # (Later loads would deadlock: their slots free only after quantize,
        # which needs the collective's result.)
        xrows = x_ext[:, :, :, :].rearrange("n c h w -> (n c) h w")
        RQ = 8 if H % 8 == 0 else 2
        n_q = H // RQ
        pre_q = min(7, n_q)
        q_tiles = []
        for rk in range(pre_q):
            xt2 = chunks.tile([128, RQ * W], f32, tag="chunk")
            nc.scalar.dma_start(xt2[:], xrows[:, rk * RQ:(rk + 1) * RQ, :])
            q_tiles.append(xt2)

        amax_p = consts.tile([128, 1], f32)
        nc.vector.tensor_reduce(
            amax_p[:], partials[:], axis=mybir.AxisListType.X,
            op=AT.max, apply_absolute_value=True)
        # reduce across partitions (standard-library Pool tensor_reduce)
        sc01 = consts.tile([128, 1], f32)  # partition 0: amax_x_local
        nc.gpsimd.tensor_reduce(
            sc01[0:1, 0:1], amax_p[:], axis=mybir.AxisListType.C, op=AT.max)

        # global amax across cores via collective
        nc.sync.dma_start(cc_in[:, :], sc01[0:1, 0:1])
        if n_cores > 1:
            nc.gpsimd.collective_compute(
                "AllReduce", AT.max,
                replica_groups=[list(range(n_cores))],
                ins=[cc_in[:, :].opt()],
                outs=[cc_out[:, :].opt()])
            cc_res = cc_out
        else:
            nc.gpsimd.dma_start(cc_out[:, :], cc_in[:, :])
            cc_res = cc_out
        gscal = consts.tile([128, 1], f32)  # p0: amax_x_global
        nc.sync.dma_start(gscal[0:1, 0:1], cc_res[:, :])

        # broadcast amax_x from partition 0 to all 128 partitions via a
        # K=1 matmul against a row of ones (standard instructions only)
        bc_ps = psump1.tile([128, 1], f32, padded_shape=[128, 512])
        nc.tensor.matmul(bc_ps[:, :], ones_row[:, :], gscal[0:1, 0:1],
                         start=True, stop=True)
        # ---------------- scales --------------------------------------------
        rax = consts.tile([128, 1], f32)
        nc.vector.reciprocal(rax[:], bc_ps[:, :])
        sx = consts.tile([128, 1], f32)   # ~127/amax_x (1/amax then *127)
        nc.vector.tensor_scalar_mul(sx[:], rax[:], 127.0)
        sw = consts.tile([128, 1], f32)   # 127/amax_w (precomputed host-side)
        nc.gpsimd.dma_start(sw[:], sw_ext[:, :])
        scale2 = consts.tile([128, 1], f32)
        nc.vector.tensor_tensor(scale2[:], sx[:], sw[:], AT.mult)
        inv = consts.tile([128, 1], f32)  # 1/(sx*sw)
        nc.vector.reciprocal(inv[:], scale2[:])
        bias_vec = consts.tile([128, 1], f32)
        nc.gpsimd.dma_start(bias_vec[0:CO, :], b_ext[:].rearrange("(o u) -> o u", u=1))
        nc.gpsimd.dma_start(bias_vec[CO:2 * CO, :], b_ext[:].rearrange("(o u) -> o u", u=1))

        # stationary weights: [ (4 image-groups x 32 ci) , (9 taps x 64 co) ]
        # quantized + transposed host-side; single contiguous DMA
        stat = statp.tile([128, 9 * CO], bf16)
        nc.gpsimd.dma_start(stat[:], qw_ext[:, :])

        # ---------------- quantize x into padded resident XQ ----------------
        # memset only the padding cells (top/bottom rows, left/right columns,
        # 2-elem tail); the interior is fully overwritten by the quantize.
        xq = xqp.tile([128, HP * WP + 2], bf16)
        xqv = xq[:, 0:HP * WP].rearrange("p (h w) -> p h w", w=WP)
        nc.vector.memset(xq[:, 0:WP], 0.0)                      # top pad row
        nc.vector.memset(xq[:, (HP - 1) * WP:HP * WP + 2], 0.0)  # bottom + tail
        nc.vector.memset(xqv[:, 1:HP - 1, 0:1], 0.0)             # left pad col
        nc.vector.memset(xqv[:, 1:HP - 1, WP - 1:WP], 0.0)       # right pad col
        for rk in range(n_q):
            if rk < pre_q:
                xt = q_tiles[rk]
            else:
                xt = chunks.tile([128, RQ * W], f32, tag="chunk")
                nc.gpsimd.dma_start(xt[:], xrows[:, rk * RQ:(rk + 1) * RQ, :])
            tmp = tmps.tile([128, RQ * W], f32)
            nc.scalar.activation(tmp[:], xt[:], AF.Copy, bias=MAGIC, scale=sx[:])
            dst = xqv[:, rk * RQ + 1:(rk + 1) * RQ + 1, 1:W + 1]
            src3 = tmp[:].rearrange("p (r w) -> p r w", w=W)
            # op2 on ScalarE too: keeping it off DVE avoids head-of-line
            # blocking of the psum epilogues (DVE is in-order)
            nc.scalar.activation(dst, src3, AF.Copy, bias=-MAGIC)

        # ---------------- conv: 9 taps x image-pair, 2-row tiles ------------
        xqf = xq  # flat [128, HP*WP+2]
        outr = out_ext[:, :, :, :].rearrange("n o h w -> (n o) h w")  # [256,H,W]
        NT = 2 * WP  # 452 moving/psum columns
        for t in range(H // 2):
            for p in range(nimg // 2):
                ps = psump.tile([128, NT], f32, padded_shape=[128, 512])
                for tap in range(9):
                    dy, dx = tap // 3, tap % 3
                    off = (2 * t + dy) * WP + dx
                    for g in range(2):
                        n = 2 * p + g
                        nc.tensor.matmul(
                            ps[64 * g:64 * (g + 1), :],
                            stat[32 * n:32 * n + 32, tap * CO:(tap + 1) * CO],
                            xqf[32 * n:32 * n + 32, off:off + NT],
                            start=(tap == 0),
                            stop=(tap == 8),
                            tile_position=(32 * n, 64 * g),
                            skip_group_check=True,
                        )
                ostage = outsp.tile([128, NT], f32)
                nc.vector.tensor_scalar(
                    ostage[:], ps[:], inv[:], bias_vec[:], AT.mult, AT.add)
                osrc = ostage[:].rearrange("p (r v) -> p r v", v=WP)[:, :, 0:W]
                nc.sync.dma_start(
                    outr[(2 * p) * CO:(2 * p + 2) * CO, 2 * t:2 * t + 2, 0:W],
                    osrc)

    nc.finalize()
    return nc


def prep_weights(weight: np.ndarray) -> dict:
    """Host-side prep of the tiny replicated weight tensor: quantize
    (identical fp32 math to the reference) and lay out as the matmul
    stationary [(4 image-groups x 32 ci), (9 taps x 64 co)] in bf16."""
    import ml_dtypes
    w = weight.astype(np.float32)
    amax_w = np.float32(np.max(np.abs(w)))
    sw = np.float32(127.0) / amax_w
    qw = np.round(w * sw)  # RNE, matches jnp.round; |qw| <= 127 exact in bf16
    qs = np.transpose(qw.reshape(64, 32, 9), (1, 2, 0)).reshape(32, 576)
    qstat = np.ascontiguousarray(np.tile(qs, (4, 1))).astype(ml_dtypes.bfloat16)
    swv = np.full((128, 1), sw, np.float32)
    return {"qw_stat": qstat, "swv": swv}


def kernel(x: np.ndarray, weight: np.ndarray, bias: np.ndarray) -> np.ndarray:
    from concourse.bass_utils import run_bass_kernel_spmd

    n_cores = 8
    N = x.shape[0]
    per = N // n_cores
    nc = build(nimg=per, H=x.shape[2], W=x.shape[3], n_cores=n_cores)
    wp = prep_weights(np.asarray(weight))
    in_maps = [
        {
            "x": np.ascontiguousarray(x[i * per:(i + 1) * per]),
            "qw_stat": wp["qw_stat"],
            "swv": wp["swv"],
            "bias": np.ascontiguousarray(bias),
        }
        for i in range(n_cores)
    ]
    res = run_bass_kernel_spmd(nc, in_maps, core_ids=list(range(n_cores)))
    outs = [r["out"] for r in res.results]
    return np.concatenate(outs, axis=0).astype(np.float32)


if __name__ == "__main__":
    # smoke: tiny build only
    nc = build(nimg=4, H=8, W=8, n_cores=2)
    print("build ok")



# revision 4
# speedup vs baseline: 1.4116x; 1.4116x over previous
"""Trainium2 Bass kernel for quantized (AdaPT int8-systolic) 3x3 Conv2d.

Reference computation (see problem):
  amax_x = max(|x|) (global), amax_w = max(|w|)
  qx = clip(round(x * 127/amax_x)), qw likewise  (integer-valued)
  out = conv2d(qx, qw, pad=1) / ((127/amax_x)*(127/amax_w)) + bias

Sharding: batch N=32 -> 4 images per core across 8 cores (data parallel),
weight/bias replicated, amax_x via AllReduce-max collective.

Per-core pipeline (single HBM pass over x):
  Phase A: stream x (fp32) in 8-row chunks; per chunk compute the abs-max
    partial (VectorE) and store the raw values as fp16 into a resident
    zero^H 1536-padded [128, 226*226+2] image in SBUF (ScalarE copy).
    fp16 keeps ~11 bits of x, enough that round(x*sx) flips only ~0.3% of
    values by +-1 (rel l2 error ~2e-3, budget 2e-2).
  amax: partition reduce + AllReduce(max) across the 8 cores.
  Phase B: quantize in place with ONE ScalarE op per slab:
    xq = fp16(x*sx + 1536)  -- the fp32->fp16 output conversion rounds to
    the nearest integer (ulp=1 in [1024,2048)), so xq holds round(x*sx)
    offset by +1536 exactly. The +1536*conv(1) bias this injects is a
    per-channel constant 1536*sum(qw) handled in the epilogue bias; the
    padding ring is memset to 1536 so the correction is exact at borders.
  Conv: 9 accumulating matmuls per (image, 4-row quad); 8 concurrent PE
    tiles of 32x64 (4 row bands = images x 2 column halves = row pairs).
    Column tiling doubles the LDWEIGHTS bandwidth, which is the limiting
    path (measured 2.46us/quad vs 4.15us with 4 tiles).
  Epilogue: out = psum*inv + (bias - 1536*sum(qw)*inv) on VectorE, written
    as bf16 (halves the output DMA; adds ~2e-3 rel error).
Integer values <= 1663 are exact in fp16 and the fp32 psum accumulation
(<2^21) is exact, so the conv matches the int32 reference bit-for-bit up
to the fp16 storage of x and the bf16 output rounding.
"""

import os
import sys
import numpy as np
from contextlib import ExitStack

sys.path.insert(0, "/opt/trn_rl_repo")

OFFS = 1536.0  # 1.5 * 2^10: fp16 ulp=1 zone; +OFFS makes fp16 output
               # conversion round x*sx to the nearest integer


def build(nimg=4, H=224, W=224, n_cores=8):
    import concourse.bass as bass
    import concourse.mybir as mybir
    import concourse.tile as tile
    from concourse import bacc

    f32 = mybir.dt.float32
    bf16 = mybir.dt.bfloat16
    fp16 = mybir.dt.float16
    CI, CO = 32, 64
    HP, WP = H + 2, W + 2
    NT = 2 * WP  # psum free size: 2 rows per column half
    assert nimg == 4 and H % 8 == 0

    nc = bacc.Bacc()
    x_ext = nc.declare_dram_parameter("x", [nimg, CI, H, W], f32, isOutput=False)
    qw_ext = nc.declare_dram_parameter("qw_stat", [128, 9 * CO], fp16,
                                       isOutput=False)
    sw_ext = nc.declare_dram_parameter("swv", [128, 1], f32, isOutput=False)
    meta_ext = nc.declare_dram_parameter("meta", [128, 2], f32, isOutput=False)
    out_ext = nc.declare_dram_parameter("out", [nimg, CO, H, W], bf16,
                                        isOutput=True)

    cc_in = nc.dram_tensor("cc_in", [1, 1], f32)
    cc_out = nc.dram_tensor("cc_out", [1, 1], f32)

    AT = mybir.AluOpType
    AF = mybir.ActivationFunctionType

    with ExitStack() as ctx:
        tc = ctx.enter_context(tile.TileContext(nc))

        consts = ctx.enter_context(tc.tile_pool(name="consts", bufs=1))
        chunks = ctx.enter_context(tc.tile_pool(name="chunks", bufs=8))
        xqp = ctx.enter_context(tc.tile_pool(name="xqp", bufs=1))
        statp = ctx.enter_context(tc.tile_pool(name="statp", bufs=1))
        psump = ctx.enter_context(tc.tile_pool(name="psum", bufs=2, space="PSUM"))
        outsp = ctx.enter_context(tc.tile_pool(name="outs", bufs=6))

        # constant-ish loads first (no deps)
        stat = statp.tile([128, 9 * CO], fp16)
        nc.gpsimd.dma_start(stat[:], qw_ext[:, :])
        sw = consts.tile([128, 1], f32)   # 127/amax_w (precomputed host-side)
        nc.gpsimd.dma_start(sw[:], sw_ext[:, :])
        meta = consts.tile([128, 2], f32)  # [:,0]=bias, [:,1]=1536*sum(qw)
        nc.gpsimd.dma_start(meta[:], meta_ext[:, :])
        ones_row = consts.tile([1, 128], f32)
        nc.vector.memset(ones_row[:], 1.0)

        # resident padded image; padding ring = OFFS (quantized-zero value)
        xq = xqp.tile([128, HP * WP + 2], fp16)
        xqv = xq[:, 0:HP * WP].rearrange("p (h w) -> p h w", w=WP)
        nc.vector.memset(xq[:, 0:WP], OFFS)                      # top pad row
        nc.vector.memset(xq[:, (HP - 1) * WP:HP * WP + 2], OFFS)  # bottom+tail
        nc.vector.memset(xqv[:, 1:HP - 1, 0:1], OFFS)             # left pad col
        nc.vector.memset(xqv[:, 1:HP - 1, WP - 1:WP], OFFS)       # right pad col

        # ---------------- Phase A: stream x, absmax + fp16 store ------------
        xrows = x_ext[:, :, :, :].rearrange("n c h w -> (n c) h w")  # [128,H,W]
        RA = 8
        n_chunks = H // RA
        partials = consts.tile([128, n_chunks], f32)
        for k in range(n_chunks):
            xt = chunks.tile([128, RA * W], f32, tag="chunk")
            eng = nc.gpsimd if (k % 2 == 0) else nc.sync
            eng.dma_start(xt[:], xrows[:, k * RA:(k + 1) * RA, :])
            nc.vector.tensor_reduce(
                partials[:, k:k + 1], xt[:], axis=mybir.AxisListType.X,
                op=AT.max, apply_absolute_value=True)
            dst = xqv[:, k * RA + 1:(k + 1) * RA + 1, 1:W + 1]
            src3 = xt[:].rearrange("p (r w) -> p r w", w=W)
            nc.scalar.activation(dst, src3, AF.Copy)  # fp32 -> fp16 raw store

        amax_p = consts.tile([128, 1], f32)
        nc.vector.tensor_reduce(
            amax_p[:], partials[:], axis=mybir.AxisListType.X,
            op=AT.max, apply_absolute_value=True)
        sc01 = consts.tile([128, 1], f32)  # partition 0: amax_x_local
        nc.gpsimd.tensor_reduce(
            sc01[0:1, 0:1], amax_p[:], axis=mybir.AxisListType.C, op=AT.max)

        # global amax across cores (cold collective; no warmup — the CC
        # stream's launch barrier ends around when phase A finishes anyway)
        nc.sync.dma_start(cc_in[:, :], sc01[0:1, 0:1])
        if n_cores > 1:
            nc.gpsimd.collective_compute(
                "AllReduce", AT.max,
                replica_groups=[list(range(n_cores))],
                ins=[cc_in[:, :].opt()],
                outs=[cc_out[:, :].opt()])
            cc_res = cc_out
        else:
            nc.gpsimd.dma_start(cc_out[:, :], cc_in[:, :])
            cc_res = cc_out
        gscal = consts.tile([128, 1], f32)  # p0: amax_x_global
        nc.sync.dma_start(gscal[0:1, 0:1], cc_res[:, :])

        # broadcast amax_x from partition 0 to all 128 partitions via a
        # K=1 matmul against a row of ones
        bc_ps = psump.tile([128, 1], f32, padded_shape=[128, 512],
                           tag="ps0", bufs=2)
        nc.tensor.matmul(bc_ps[:, :], ones_row[:, :], gscal[0:1, 0:1],
                         start=True, stop=True)
        # ---------------- scales --------------------------------------------
        rax = consts.tile([128, 1], f32)
        nc.vector.reciprocal(rax[:], bc_ps[:, :])
        sx = consts.tile([128, 1], f32)   # ~127/amax_x (1/amax then *127)
        nc.vector.tensor_scalar_mul(sx[:], rax[:], 127.0)
        scale2 = consts.tile([128, 1], f32)
        nc.vector.tensor_tensor(scale2[:], sx[:], sw[:], AT.mult)
        inv = consts.tile([128, 1], f32)  # 1/(sx*sw)
        nc.vector.reciprocal(inv[:], scale2[:])
        bias_vec = consts.tile([128, 1], f32)  # bias - 1536*sum(qw)*inv
        nc.vector.tensor_tensor(bias_vec[:], meta[:, 1:2], inv[:], AT.mult)
        nc.vector.tensor_tensor(bias_vec[:], meta[:, 0:1], bias_vec[:],
                                AT.subtract)

        # ---------------- Phase B: in-place quantize ------------------------
        # xq = fp16(x*sx + 1536): integer-valued + 1536, exact in fp16
        for rk in range(n_chunks):
            sl = xqv[:, rk * RA + 1:(rk + 1) * RA + 1, 1:W + 1]
            nc.scalar.activation(sl, sl, AF.Copy, bias=OFFS, scale=sx[:])

        # ---------------- conv: 8-way tiled, 4-row quads --------------------
        nq = H // 4
        for q in range(nq):
            pss = []
            for n in range(nimg):
                pst = psump.tile([128, NT], f32, padded_shape=[128, 512],
                                 name=f"ps_{q}_{n}", tag=f"ps{n}", bufs=2)
                pss.append(pst)
            for tap in range(9):
                dy, dx = tap // 3, tap % 3
                for n in range(nimg):
                    for c in range(2):
                        off = (4 * q + 2 * c + dy) * WP + dx
                        nc.tensor.matmul(
                            pss[n][64 * c:64 * (c + 1), :],
                            stat[32 * n:32 * n + 32, tap * CO:(tap + 1) * CO],
                            xq[32 * n:32 * n + 32, off:off + NT],
                            start=(tap == 0), stop=(tap == 8),
                            tile_position=(32 * n, 64 * c),
                            skip_group_check=True)
            for n in range(nimg):
                ostage = outsp.tile([128, NT], bf16, tag="ost", bufs=6)
                nc.vector.tensor_scalar(
                    ostage[:], pss[n][:], inv[:], bias_vec[:], AT.mult, AT.add)
                osrc = ostage[:].rearrange("p (r v) -> p r v", v=WP)[:, :, 0:W]
                for hb in range(2):
                    eng = nc.sync if hb == 0 else nc.gpsimd
                    eng.dma_start(
                        out_ext[n, :, 4 * q + 2 * hb:4 * q + 2 * hb + 2, :],
                        osrc[64 * hb:64 * hb + 64])

    nc.finalize()
    return nc


def prep_weights(weight: np.ndarray, bias: np.ndarray) -> dict:
    """Host-side prep of the tiny replicated weight tensor: quantize
    (identical fp32 math to the reference), lay out as the matmul
    stationary [(4 image-bands x 32 ci), (9 taps x 64 co)] in fp16, and
    fold bias + the +1536 quantize-offset correction into meta."""
    w = weight.astype(np.float32)
    amax_w = np.float32(np.max(np.abs(w)))
    sw = np.float32(127.0) / amax_w
    qw = np.round(w * sw)  # RNE, matches jnp.round; |qw| <= 127 exact in fp16
    qs = np.transpose(qw.reshape(64, 32, 9), (1, 2, 0)).reshape(32, 576)
    qstat = np.ascontiguousarray(np.tile(qs, (4, 1))).astype(np.float16)
    swv = np.full((128, 1), sw, np.float32)
    ko = qw.sum(axis=(1, 2, 3)).astype(np.float32)  # sum over ci,dy,dx
    meta = np.stack([np.tile(bias.astype(np.float32), 2),
                     np.tile(np.float32(OFFS) * ko, 2)], axis=1)
    meta = np.ascontiguousarray(meta, np.float32)
    return {"qw_stat": qstat, "swv": swv, "meta": meta}


def kernel(x: np.ndarray, weight: np.ndarray, bias: np.ndarray) -> np.ndarray:
    from concourse.bass_utils import run_bass_kernel_spmd

    n_cores = 8
    N = x.shape[0]
    per = N // n_cores
    nc = build(nimg=per, H=x.shape[2], W=x.shape[3], n_cores=n_cores)
    wp = prep_weights(np.asarray(weight), np.asarray(bias))
    in_maps = [
        {
            "x": np.ascontiguousarray(x[i * per:(i + 1) * per]),
            "qw_stat": wp["qw_stat"],
            "swv": wp["swv"],
            "meta": wp["meta"],
        }
        for i in range(n_cores)
    ]
    res = run_bass_kernel_spmd(nc, in_maps, core_ids=list(range(n_cores)))
    outs = [np.asarray(r["out"]).astype(np.float32) for r in res.results]
    return np.concatenate(outs, axis=0)


if __name__ == "__main__":
    # smoke: tiny build only
    nc = build(nimg=4, H=8, W=8, n_cores=2)
    print("build ok")
